# revision 1
# baseline (speedup 1.0000x reference)
"""EnsembleGRU Trainium2 kernel.

Math (per ensemble member e, H=1):
    y  = x @ Wl^T + bl                      (proj)
    gi = y @ Wih^T + bih                    -> fold: gi = x @ Wc^T + bc
         Wc = Wih @ Wl   (3,8),  bc = Wih @ bl + bih (+ bhh for r,z gates)
    scan over W steps:
        r  = sigmoid(gi_r + a*h)            a = whh[0]
        z  = sigmoid(gi_z + b*h)            b = whh[1]
        n  = tanh(gi_n + r*(c*h + d))       c = whh[2], d = bhh[2]
        h' = (1-z)*n + z*h = z*h - (z-1)*n

Sharding: E=16 members over 8 cores (2 per core), zero communication.
Lane layout per core: partition p = e_loc*64 + p' (p' in 0..63),
free col c in 0..39, bi = p'*40 + c  (5120 lanes = 128 x 40).

gi is computed on the TensorEngine with x in its *natural* layout:
gi_g[p, (w,c)] = sum_f Wc[e(p),g,f] * x[p, (w,c,f)] via 8 accumulating
diagonal matmuls (one per f) + 1 bias matmul against a ones tile.
The 64-step scan runs on DVE (fused scalar_tensor_tensor) + ACT
(exact Sigmoid/Tanh LUTs), with gi read directly from PSUM.
"""

import numpy as np

W, E, B, I, F = 64, 16, 256, 10, 8
BI = B * I            # 2560
NCORES = 8
E_LOC = E // NCORES   # 2
PP = 64               # partitions per member
CC = BI // PP         # 40 free cols per step
G = 3                 # gates

# w-group sizes for gi matmul tiling (PSUM: 3 banks per group, double buffered)
WGROUPS = [8] * 8
assert sum(WGROUPS) == W
NDIAG = 27  # 24 (g,f) Wc diags + 3 bias diags

_CACHED = {}


def _build_nc(d_nonzero: bool, rep: int = 1, mm_only: bool = False, scan_only: bool = False):
    import contextlib

    import concourse.bacc as bacc
    import concourse.mybir as mybir
    from concourse.tile import TileContext

    AL = mybir.AluOpType
    AF = mybir.ActivationFunctionType
    f32 = mybir.dt.float32
    f16 = mybir.dt.float16

    nc = bacc.Bacc("TRN2", target_bir_lowering=False)

    xh = nc.dram_tensor("xh", [128, F, W, CC], f16, kind="ExternalInput")
    dg = nc.dram_tensor("dg", [128, NDIAG * 128], f16, kind="ExternalInput")
    cst = nc.dram_tensor("cst", [128, 7 + CC], f32, kind="ExternalInput")
    out = nc.dram_tensor("out", [128, W * CC], f32, kind="ExternalOutput")

    with TileContext(nc) as tc:
        with (
            tc.tile_pool(name="const", bufs=1) as constp,
            tc.tile_pool(name="xp", bufs=2) as xp,
            tc.tile_pool(name="gip", bufs=2, space="PSUM") as gip,
            tc.tile_pool(name="app", bufs=2, space="PSUM") as app,
            tc.tile_pool(name="scan", bufs=3) as scanp,
            tc.tile_pool(name="outp", bufs=1) as outp,
        ):
            dg_sb = constp.tile([128, NDIAG * 128], f16, tag="dg")
            cst_sb = constp.tile([128, 7 + CC], f32, tag="cst")
            ones = constp.tile([128, 12 * CC], f16, tag="ones")
            out_sb = outp.tile([128, (W + 1) * CC], f32, tag="out")

            nc.sync.dma_start(dg_sb[:], dg[:])
            nc.sync.dma_start(cst_sb[:], cst[:])
            nc.vector.memset(ones[:], 1.0)
            # h0 into slot 0
            nc.vector.tensor_copy(out_sb[:, 0:CC], cst_sb[:, 7 : 7 + CC])

            a_s = cst_sb[:, 0:1]
            b_s = cst_sb[:, 1:2]
            c_s = cst_sb[:, 2:3]
            d_s = cst_sb[:, 3:4]
            bn_s = cst_sb[:, 4:5]
            na_s = cst_sb[:, 5:6]  # -a
            nb_s = cst_sb[:, 6:7]  # -b

            loop_cm = tc.For_i(0, rep, 1) if rep > 1 else contextlib.nullcontext()
            with loop_cm:
                _body(
                    nc, tc, xp, gip, app, scanp, xh, out, dg_sb, cst_sb, ones, out_sb,
                    a_s, b_s, c_s, d_s, bn_s, na_s, nb_s, AL, AF, f32, f16,
                    d_nonzero, mm_only, scan_only,
                )

    nc.finalize()
    return nc


def _body(
    nc, tc, xp, gip, app, scanp, xh, out, dg_sb, cst_sb, ones, out_sb,
    a_s, b_s, c_s, d_s, bn_s, na_s, nb_s, AL, AF, f32, f16,
    d_nonzero, mm_only, scan_only,
):
    ngrp = len(WGROUPS)
    gstart = [sum(WGROUPS[:k]) for k in range(ngrp)]
    gi_tiles = {}

    def emit_group(k):
        WG = WGROUPS[k]
        w0 = gstart[k]
        x_t = xp.tile([128, F * WG * CC], f16, tag="x")
        nc.sync.dma_start(
            x_t[:].rearrange("p (f w c) -> p f w c", f=F, c=CC),
            xh[:, :, w0 : w0 + WG, :],
        )
        gi_ps = gip.tile([128, 3 * 512], f32, tag="gi")
        gi_tiles[k] = gi_ps
        if not scan_only:
            for g in range(G):
                reg = gi_ps[:, g * 512 : g * 512 + WG * CC]
                # bias first for r/z gates (start=True clears bank region);
                # n-gate bias is folded into the scan's `an` op instead.
                if g < 2:
                    nc.tensor.matmul(
                        reg,
                        dg_sb[:, (24 + g) * 128 : (25 + g) * 128],
                        ones[:, : WG * CC],
                        start=True,
                        stop=False,
                        skip_group_check=True,
                    )
                for f in range(F):
                    # contiguous (WG*CC)-wide rhs slab per (g, f)
                    nc.tensor.matmul(
                        reg,
                        dg_sb[:, (g * F + f) * 128 : (g * F + f + 1) * 128],
                        x_t[:, f * WG * CC : (f + 1) * WG * CC],
                        start=(g == 2 and f == 0),
                        stop=(f == F - 1),
                        skip_group_check=True,
                    )
        else:
            # init psum regions so the scan's reads have a producer
            for g in range(G):
                nc.tensor.matmul(
                    gi_ps[:, g * 512 : g * 512 + WG * CC],
                    dg_sb[:, (24 + g) * 128 : (25 + g) * 128],
                    ones[:, : WG * CC],
                    start=True,
                    stop=True,
                    skip_group_check=True,
                )

    def gi_ap(w, g):
        k = 0
        while k + 1 < ngrp and w >= gstart[k + 1]:
            k += 1
        wl = w - gstart[k]
        return gi_tiles[k][:, g * 512 + wl * CC : g * 512 + (wl + 1) * CC]

    emit_group(0)
    if ngrp > 1:
        emit_group(1)

    # scan — software-pipelined: ar/az for step w+1 are rebuilt from
    # (q, u) of step w (h' = q - u) so the next sigmoid's inputs are
    # ready one DVE-op earlier:  ar(w+1) = -a*u - P1',
    # P1' = -(gi_r(w+1) + a*q)  computed while tanh(w) runs.
    def emit_out_dma(k):
        nc.sync.dma_start(
            out[:, gstart[k] * CC : (gstart[k] + WGROUPS[k]) * CC],
            out_sb[:, (gstart[k] + 1) * CC : (gstart[k] + WGROUPS[k] + 1) * CC],
        )

    # group-end step -> group idx (last group's DMA is emitted after the loop)
    gends = {gstart[k] + WGROUPS[k] - 1: k for k in range(ngrp - 1)}

    if mm_only:
        for k in range(2, ngrp):
            emit_group(k)
    else:
        u_prev = None
        p1_prev = None
        q_prev = None
        for w in range(W):
            h = out_sb[:, w * CC : (w + 1) * CC]

            aa = app.tile([128, 3 * CC], f32, tag="aa")  # [ar|az|an] in PSUM
            rz = scanp.tile([128, 2 * CC], f32, tag="rz")
            v = scanp.tile([128, CC], f32, tag="v")
            n_t = scanp.tile([128, CC], f32, tag="n")
            u = scanp.tile([128, CC], f32, tag="u")
            q = scanp.tile([128, CC], f32, tag="q")
            p1 = scanp.tile([128, 2 * CC], f32, tag="p1")

            if w == 0:
                nc.vector.scalar_tensor_tensor(
                    aa[:, 0:CC], h, a_s, gi_ap(0, 0), AL.mult, AL.add
                )
                nc.vector.scalar_tensor_tensor(
                    aa[:, CC : 2 * CC], h, b_s, gi_ap(0, 1), AL.mult, AL.add
                )
            else:
                # ar = (u*-a) - P1'_r ; az = (u*-b) - P1'_z
                nc.vector.scalar_tensor_tensor(
                    aa[:, 0:CC], u_prev, na_s, p1_prev[:, 0:CC], AL.mult, AL.subtract
                )
                nc.vector.scalar_tensor_tensor(
                    aa[:, CC : 2 * CC], u_prev, nb_s, p1_prev[:, CC:], AL.mult,
                    AL.subtract,
                )
                # deferred h'(w-1) = q - u: lands in the sigmoid's shadow,
                # off the DVE chain between u(w-1) and ar(w)
                nc.vector.tensor_tensor(h, q_prev, u_prev, AL.subtract)
                if w >= 1 and (w - 1) in gends:
                    emit_out_dma(gends[w - 1])
            nc.scalar.activation(rz[:], aa[:, 0 : 2 * CC], AF.Sigmoid)
            # v = c*h*r  (+ d*r if d != 0)
            nc.vector.scalar_tensor_tensor(
                v[:], h, c_s, rz[:, 0:CC], AL.mult, AL.mult
            )
            if d_nonzero:
                nc.vector.scalar_tensor_tensor(
                    v[:], rz[:, 0:CC], d_s, v[:], AL.mult, AL.add
                )
            # an = (gi_n + bc_n) + v   (n-gate bias folded here)
            nc.vector.scalar_tensor_tensor(
                aa[:, 2 * CC :], gi_ap(w, 2), bn_s, v[:], AL.add, AL.add
            )
            # q = z*h, then prefetch P1' for the next step (overlaps tanh)
            nc.vector.tensor_tensor(q[:], rz[:, CC:], h, AL.mult)
            if w + 1 < W:
                nc.vector.scalar_tensor_tensor(
                    p1[:, 0:CC], q[:], na_s, gi_ap(w + 1, 0), AL.mult, AL.subtract
                )
                nc.vector.scalar_tensor_tensor(
                    p1[:, CC:], q[:], nb_s, gi_ap(w + 1, 1), AL.mult, AL.subtract
                )
            nc.scalar.activation(n_t[:], aa[:, 2 * CC :], AF.Tanh)
            # u = (z-1)*n ; h' = q - u
            nc.vector.scalar_tensor_tensor(
                u[:], rz[:, CC:], 1.0, n_t[:], AL.subtract, AL.mult
            )
            u_prev, p1_prev, q_prev = u, p1, q

            # interleave: after the first step of group k, emit group k+2's
            # DMA + matmuls so PE/DMA work schedules under this group's scan
            k = 0
            while k + 1 < ngrp and w >= gstart[k + 1]:
                k += 1
            if w == gstart[k] and k + 2 < ngrp:
                emit_group(k + 2)

        # final h' and last group's output
        nc.vector.tensor_tensor(
            out_sb[:, W * CC : (W + 1) * CC], q_prev, u_prev, AL.subtract
        )
        emit_out_dma(ngrp - 1)


def _prep_core_inputs(inputs, core):
    x = inputs["inputs"]          # (W,E,B,I,F) f32
    state = inputs["state"]       # (1,E,BI,1)
    wl = inputs["weight_linear"]  # (E,16,F)
    bl = inputs["bias_linear"]    # (E,16)
    wih = inputs["weight_ih"]     # (E,3,16)
    whh = inputs["weight_hh"]     # (E,3,1)
    bih = inputs["bias_ih"]       # (E,3)
    bhh = inputs["bias_hh"]       # (E,3)

    es = slice(core * E_LOC, (core + 1) * E_LOC)
    # fold weights
    Wc = np.einsum("egp,epf->egf", wih[es], wl[es])          # (2,3,F)
    bc = np.einsum("egp,ep->eg", wih[es], bl[es]) + bih[es]  # (2,3)
    bc = bc.copy()
    bc[:, 0] += bhh[es][:, 0]
    bc[:, 1] += bhh[es][:, 1]

    # x -> (128, F, W, CC) fp16 (f-major so matmul rhs slabs are contiguous)
    xr = np.asarray(x[:, es]).reshape(W, E_LOC, PP, CC, F)
    xh = np.ascontiguousarray(xr.transpose(1, 2, 4, 0, 3)).reshape(128, F, W, CC)
    xh = xh.astype(np.float16)

    # diags (128, 27, 128) fp16
    pe = np.repeat(np.arange(E_LOC), PP)  # (128,) member index per partition
    dgv = np.zeros((128, NDIAG), np.float32)
    for g in range(G):
        for f in range(F):
            dgv[:, g * F + f] = Wc[pe, g, f]
        dgv[:, 24 + g] = bc[pe, g]
    dg = np.zeros((128, NDIAG, 128), np.float16)
    idx = np.arange(128)
    dg[idx, :, idx] = dgv.astype(np.float16)
    dg = dg.reshape(128, NDIAG * 128)

    # consts (128, 7+CC) f32
    cstv = np.zeros((128, 7 + CC), np.float32)
    cstv[:, 0] = whh[es][pe, 0, 0]
    cstv[:, 1] = whh[es][pe, 1, 0]
    cstv[:, 2] = whh[es][pe, 2, 0]
    cstv[:, 3] = bhh[es][pe, 2]
    cstv[:, 4] = bc[pe, 2]  # n-gate bias, folded into scan
    cstv[:, 5] = -cstv[:, 0]
    cstv[:, 6] = -cstv[:, 1]
    h0 = np.asarray(state[-1, es, :, 0]).reshape(E_LOC, PP, CC)
    cstv[:, 7:] = h0.reshape(128, CC)

    return {"xh": xh, "dg": dg, "cst": cstv}


def kernel(**inputs):
    from concourse.bass_utils import run_bass_kernel_spmd

    bhh = np.asarray(inputs["bias_hh"])
    d_nonzero = bool(np.any(bhh[:, 2] != 0))

    key = ("nc", d_nonzero)
    if key not in _CACHED:
        _CACHED[key] = _build_nc(d_nonzero)
    nc = _CACHED[key]

    in_maps = [_prep_core_inputs(inputs, c) for c in range(NCORES)]
    res = run_bass_kernel_spmd(nc, in_maps, core_ids=list(range(NCORES)))

    # reassemble: per-core out (128, W*CC) -> (W, E_LOC, BI)
    full = np.zeros((W, E, B, I, 1), np.float32)
    for c in range(NCORES):
        o = np.asarray(res.results[c]["out"]).reshape(E_LOC, PP, W, CC)
        o = o.transpose(2, 0, 1, 3).reshape(W, E_LOC, BI)
        full[:, c * E_LOC : (c + 1) * E_LOC] = o.reshape(W, E_LOC, B, I, 1)
    return full



# revision 19
# speedup vs baseline: 4.8912x; 4.8912x over previous
"""EnsembleGRU Trainium2 kernel (v2).

Math (per ensemble member e, H=1):
    y  = x @ Wl^T + bl                      (proj)
    gi = y @ Wih^T + bih                    -> fold: gi = x @ Wc^T + bc
         Wc = Wih @ Wl   (3,8),  bc = Wih @ bl + bih (+ bhh for r,z gates)
    scan over W steps:
        r  = sigmoid(gi_r + a*h)            a = whh[0]
        z  = sigmoid(gi_z + b*h)            b = whh[1]
        n  = tanh(gi_n + r*(c*h + d))       c = whh[2], d = bhh[2]
        h' = (1-z)*n + z*h = q - u,  q = z*h,  u = (z-1)*n

Sharding: E=16 members over 8 cores (2 per core), zero communication.
Lane layout per core: partition p = e_loc*64 + p' (p' in 0..63),
free col c in 0..39, bi = p'*40 + c  (5120 lanes = 128 x 40).

gi is computed on the TensorEngine via accumulating diagonal matmuls
(per (gate,f) diag + per-gate bias diag vs a ones tile), into PSUM.
x is staged w-major ("p (w f c)") so each group's DMA is fully
contiguous on both sides; matmul rhs uses a strided [w, c] view.

The 64-step scan runs on DVE + ACT only.  Every DVE op is an
immediate-scalar scalar_tensor_tensor on SBUF operands (the cheapest
primitive); per-member constants (a, b, c, -a, -b, d) enter as
stride-0 broadcast in1 APs of a per-core const tile, keeping the
program SPMD-uniform.  The sigmoid is split into two [128,40] calls
(gates r and z), tanh is one [128,40] call.  Next-step gate inputs are
rebuilt from (q, u) with the P-prefetch trick so only u -> -a*u -> +P
sits on the post-tanh critical path:
    P_r(w+1) = a*q(w) + gi_r(w+1)           (in tanh's shadow)
    ar(w+1)  = (-a)*u(w) + P_r(w+1)
    h(w)     = q(w) - u(w)                  (deferred, in sigmoid's shadow)
"""

import numpy as np

W, E, B, I, F = 64, 16, 256, 10, 8
BI = B * I            # 2560
NCORES = 8
E_LOC = E // NCORES   # 2
PP = 64               # partitions per member
CC = BI // PP         # 40 free cols per step
G = 3                 # gates

WGROUPS = [8] * 8
assert sum(WGROUPS) == W
NDIAG = 27  # 24 (g,f) Wc diags + 3 bias diags

_CACHED = {}


def _build_nc(d_nonzero: bool, rep: int = 1, mm_only: bool = False,
              scan_only: bool = False, no_xdma: bool = False, act_dma: bool = False,
              dve_pad: int = 0, act_pad: int = 0, sigma_one: bool = True, aa_pad: int = 48,
              za_trick: bool = True, pad_each: bool = True,
              all_sigmoid: bool = False, pad_whole: bool = False):
    import contextlib

    import concourse.bacc as bacc
    import concourse.mybir as mybir
    from concourse.tile import TileContext

    AL = mybir.AluOpType
    AF = mybir.ActivationFunctionType
    f32 = mybir.dt.float32
    f16 = mybir.dt.float16

    nc = bacc.Bacc("TRN2", target_bir_lowering=False)

    xh = nc.dram_tensor("xh", [128, W * F * CC], f16, kind="ExternalInput")
    dg = nc.dram_tensor("dg", [128, NDIAG * 128], f16, kind="ExternalInput")
    cst = nc.dram_tensor("cst", [128, 6 + CC], f32, kind="ExternalInput")
    out = nc.dram_tensor("out", [128, W * CC], f32, kind="ExternalOutput")

    with TileContext(nc) as tc:
        with (
            tc.tile_pool(name="const", bufs=1) as constp,
            tc.tile_pool(name="xp", bufs=2) as xp,
            tc.tile_pool(name="gip", bufs=2, space="PSUM") as gip,
            tc.tile_pool(name="scan", bufs=3) as scanp,
            tc.tile_pool(name="outp", bufs=1) as outp,
        ):
            dg_sb = constp.tile([128, NDIAG * 128], f16, tag="dg")
            cst_sb = constp.tile([128, 6 + CC], f32, tag="cst")
            ones = constp.tile([128, max(WGROUPS) * CC], f16, tag="ones")
            out_sb = outp.tile([128, (W + 1) * CC], f32, tag="out")

            nc.sync.dma_start(dg_sb[:], dg[:])
            nc.sync.dma_start(cst_sb[:], cst[:])
            nc.vector.memset(ones[:], 1.0)
            # h0 into slot 0
            nc.vector.tensor_copy(out_sb[:, 0:CC], cst_sb[:, 6 : 6 + CC])

            bc = [128, CC]
            a_bc = cst_sb[:, 0:1].broadcast_to(bc)
            b_bc = cst_sb[:, 1:2].broadcast_to(bc)
            c_bc = cst_sb[:, 2:3].broadcast_to(bc)
            na_bc = cst_sb[:, 3:4].broadcast_to(bc)
            nb_bc = cst_sb[:, 4:5].broadcast_to(bc)
            d_bc = cst_sb[:, 5:6].broadcast_to(bc)
            bc2 = [128, 2, CC]
            ab_bc = cst_sb[:, 0:2].rearrange("p (g c) -> p g c", c=1).broadcast_to(bc2)
            nanb_bc = cst_sb[:, 3:5].rearrange("p (g c) -> p g c", c=1).broadcast_to(bc2)

            loop_cm = tc.For_i(0, rep, 1) if rep > 1 else contextlib.nullcontext()
            with loop_cm:
                _body(
                    nc, tc, xp, gip, scanp, xh, out, dg_sb, ones, out_sb,
                    a_bc, b_bc, c_bc, na_bc, nb_bc, d_bc, ab_bc, nanb_bc,
                    AL, AF, f32, f16,
                    d_nonzero, mm_only, scan_only, no_xdma, act_dma,
                    dve_pad, act_pad, sigma_one, all_sigmoid, pad_whole, aa_pad,
                    za_trick, pad_each,
                )

    nc.finalize()
    return nc


def _body(
    nc, tc, xp, gip, scanp, xh, out, dg_sb, ones, out_sb,
    a_bc, b_bc, c_bc, na_bc, nb_bc, d_bc, ab_bc, nanb_bc, AL, AF, f32, f16,
    d_nonzero, mm_only, scan_only, no_xdma=False, act_dma=False,
    dve_pad=0, act_pad=0, sigma_one=True, all_sigmoid=False, pad_whole=False,
    aa_pad=48, za_trick=True, pad_each=True,
):
    ngrp = len(WGROUPS)
    gstart = [sum(WGROUPS[:k]) for k in range(ngrp)]
    gi_tiles = {}

    def emit_group(k):
        WG = WGROUPS[k]
        w0 = gstart[k]
        x_t = xp.tile([128, WG * F * CC], f16, tag="x")
        if not no_xdma:
            eng = nc.scalar if (act_dma and k % 2) else nc.sync
            eng.dma_start(x_t[:], xh[:, w0 * F * CC : (w0 + WG) * F * CC])
        else:
            nc.vector.memset(x_t[:], 0.001)
        gi_ps = gip.tile([128, 3 * 512], f32, tag="gi")
        gi_tiles[k] = gi_ps
        xv = x_t[:].rearrange("p (w f c) -> p w f c", w=WG, f=F)
        if not scan_only:
            for g in range(G):
                reg = gi_ps[:, g * 512 : g * 512 + WG * CC]
                regv = reg.rearrange("p (w c) -> p w c", c=CC)
                nc.tensor.matmul(
                    reg,
                    dg_sb[:, (24 + g) * 128 : (25 + g) * 128],
                    ones[:, : WG * CC],
                    start=True,
                    stop=False,
                    skip_group_check=True,
                )
                for f in range(F):
                    nc.tensor.matmul(
                        regv,
                        dg_sb[:, (g * F + f) * 128 : (g * F + f + 1) * 128],
                        xv[:, :, f, :],
                        start=False,
                        stop=(f == F - 1),
                        skip_group_check=True,
                    )
        else:
            for g in range(G):
                nc.tensor.matmul(
                    gi_ps[:, g * 512 : g * 512 + WG * CC],
                    dg_sb[:, (24 + g) * 128 : (25 + g) * 128],
                    ones[:, : WG * CC],
                    start=True,
                    stop=True,
                    skip_group_check=True,
                )

    def gi_ap(w, g):
        k = 0
        while k + 1 < ngrp and w >= gstart[k + 1]:
            k += 1
        wl = w - gstart[k]
        return gi_tiles[k][:, g * 512 + wl * CC : g * 512 + (wl + 1) * CC]

    def gi_rz_ap(w):
        # [128, 2, CC] view of gates r,z for step w (gate stride 512)
        k = 0
        while k + 1 < ngrp and w >= gstart[k + 1]:
            k += 1
        wl = w - gstart[k]
        g3 = gi_tiles[k][:].rearrange("p (g x) -> p g x", g=3)
        return g3[:, 0:2, wl * CC : (wl + 1) * CC]

    emit_group(0)
    if ngrp > 1:
        emit_group(1)

    def emit_out_dma(k):
        nc.sync.dma_start(
            out[:, gstart[k] * CC : (gstart[k] + WGROUPS[k]) * CC],
            out_sb[:, (gstart[k] + 1) * CC : (gstart[k] + WGROUPS[k] + 1) * CC],
        )

    gends = {gstart[k] + WGROUPS[k] - 1: k for k in range(ngrp - 1)}

    if mm_only:
        for k in range(2, ngrp):
            emit_group(k)
        return

    stt = nc.vector.scalar_tensor_tensor

    # bootstrap: aa(0) = [ar|az](0) from h0, ch(0)
    AAW = 2 * CC + aa_pad
    h0 = out_sb[:, 0:CC]
    aa = scanp.tile([128, AAW], f32, tag="aa", name="aa_boot")
    ch = scanp.tile([128, CC], f32, tag="ch", name="ch_boot")
    t0 = scanp.tile([128, 2 * CC], f32, tag="t0", name="t0_boot")
    stt(t0[:, 0:CC], h0, 1.0, a_bc, AL.mult, AL.mult)
    stt(t0[:, CC:], h0, 1.0, b_bc, AL.mult, AL.mult)
    stt(aa[:, 0:CC], t0[:, 0:CC], 0.0, gi_ap(0, 0), AL.add, AL.add)
    stt(aa[:, CC : 2 * CC], t0[:, CC:], 0.0, gi_ap(0, 1), AL.add, AL.add)
    if aa_pad:
        # zero the pad lanes of this pool slot once; rotation preserves them
        nc.vector.memset(aa[:, 2 * CC :], 0.0)
    stt(ch[:], h0, 1.0, c_bc, AL.mult, AL.mult)
    if d_nonzero:
        stt(ch[:], ch[:], 0.0, d_bc, AL.add, AL.add)

    q_prev = None
    u_prev = None
    for w in range(W):
        h = out_sb[:, w * CC : (w + 1) * CC]

        rz = scanp.tile([128, AAW], f32, tag="rz")
        r_t = rz[:, 0:CC]
        z_t = rz[:, CC : 2 * CC]
        v = scanp.tile([128, CC], f32, tag="v")
        an_t = scanp.tile([128, CC], f32, tag="an")
        n_t = scanp.tile([128, CC], f32, tag="n")
        u = scanp.tile([128, CC], f32, tag="u")
        q = scanp.tile([128, CC], f32, tag="q")

        # sigmoid gates: one whole-tile call over [ar|az]
        nc.scalar.activation(rz[:], aa[:], AF.Sigmoid)
        if dve_pad or act_pad:
            pad_t = scanp.tile([128, CC], f32, tag="pad", name=f"pad_{w}")
            for _ in range(dve_pad):
                stt(pad_t[:], r_t, 1.0, a_bc, AL.mult, AL.mult)
            for _ in range(act_pad):
                nc.scalar.activation(pad_t[:], pad_t[:], AF.Sigmoid)

        # deferred h(w-1) = q - u, lands in sigmoid's shadow; ch = c*h follows
        if w >= 1:
            stt(h, q_prev, 0.0, u_prev, AL.add, AL.subtract)
            if (w - 1) in gends:
                emit_out_dma(gends[w - 1])
            ch = scanp.tile([128, CC], f32, tag="ch", name=f"ch_{w}")
            stt(ch[:], h, 1.0, c_bc, AL.mult, AL.mult)
            if d_nonzero:
                stt(ch[:], ch[:], 0.0, d_bc, AL.add, AL.add)

        # chain: v = ch * r ; an = v + gi_n(w)
        stt(v[:], ch[:], 1.0, r_t, AL.mult, AL.mult)
        stt(an_t[:], v[:], 0.0, gi_ap(w, 2), AL.add, AL.add)

        # tanh-shadow ops: q, P_r = a*q + gi_r(w+1), P_z = b*q + gi_z(w+1)
        stt(q[:], z_t, 1.0, h, AL.mult, AL.mult)
        if za_trick and w + 1 < W:
            zab = scanp.tile([128, 2 * CC], f32, tag="zab", name=f"zab_{w}")
            stt(zab[:, 0:CC], z_t, 1.0, na_bc, AL.subtract, AL.mult)
            stt(zab[:, CC:], z_t, 1.0, nb_bc, AL.subtract, AL.mult)
        if w + 1 < W:
            aa_n = scanp.tile([128, AAW], f32, tag="aa", name=f"aa_{w}")
            pp_t = scanp.tile([128, 2 * CC], f32, tag="pp", name=f"pp_{w}")
            t_n = scanp.tile([128, 2 * CC], f32, tag="t0", name=f"t_{w}")
            stt(t_n[:, 0:CC], q[:], 1.0, a_bc, AL.mult, AL.mult)
            stt(pp_t[:, 0:CC], t_n[:, 0:CC], 0.0, gi_ap(w + 1, 0), AL.add, AL.add)
            stt(t_n[:, CC:], q[:], 1.0, b_bc, AL.mult, AL.mult)
            stt(pp_t[:, CC:], t_n[:, CC:], 0.0, gi_ap(w + 1, 1), AL.add, AL.add)

        nc.scalar.activation(n_t[:], an_t[:],
                             AF.Sigmoid if all_sigmoid else AF.Tanh)

        # chain: u = (z-1)*n ; ar = -a*u + P_r ; az = -b*u + P_z
        stt(u[:], z_t, 1.0, n_t[:], AL.subtract, AL.mult)
        if w + 1 < W:
            if za_trick:
                # -a*u = ((z-1)*(-a)) * n, computed straight from n
                stt(t_n[:, 0:CC], n_t[:], 1.0, zab[:, 0:CC], AL.mult, AL.mult)
            else:
                stt(t_n[:, 0:CC], u[:], 1.0, na_bc, AL.mult, AL.mult)
            stt(aa_n[:, 0:CC], t_n[:, 0:CC], 0.0, pp_t[:, 0:CC], AL.add, AL.add)
            if za_trick:
                stt(t_n[:, CC:], n_t[:], 1.0, zab[:, CC:], AL.mult, AL.mult)
            else:
                stt(t_n[:, CC:], u[:], 1.0, nb_bc, AL.mult, AL.mult)
            stt(aa_n[:, CC : 2 * CC], t_n[:, CC:], 0.0, pp_t[:, CC:], AL.add, AL.add)
            if aa_pad and (pad_each or w < 2):
                nc.vector.memset(aa_n[:, 2 * CC :], 0.0)
            aa = aa_n

        q_prev, u_prev = q, u

        # interleave: after the first step of group k, emit group k+2
        k = 0
        while k + 1 < ngrp and w >= gstart[k + 1]:
            k += 1
        if w == gstart[k] and k + 2 < ngrp:
            emit_group(k + 2)

    # final h(W-1) and last group's output
    stt(out_sb[:, W * CC : (W + 1) * CC], q_prev, 0.0, u_prev, AL.add, AL.subtract)
    emit_out_dma(ngrp - 1)


def _prep_core_inputs(inputs, core):
    x = inputs["inputs"]          # (W,E,B,I,F) f32
    state = inputs["state"]       # (1,E,BI,1)
    wl = inputs["weight_linear"]  # (E,16,F)
    bl = inputs["bias_linear"]    # (E,16)
    wih = inputs["weight_ih"]     # (E,3,16)
    whh = inputs["weight_hh"]     # (E,3,1)
    bih = inputs["bias_ih"]       # (E,3)
    bhh = inputs["bias_hh"]       # (E,3)

    es = slice(core * E_LOC, (core + 1) * E_LOC)
    # fold weights
    Wc = np.einsum("egp,epf->egf", wih[es], wl[es])          # (2,3,F)
    bc = np.einsum("egp,ep->eg", wih[es], bl[es]) + bih[es]  # (2,3)
    bc = bc.copy()
    bc[:, 0] += bhh[es][:, 0]
    bc[:, 1] += bhh[es][:, 1]
    # gate-n bias (bc[:,2]) included in the bias diag matmul; d=bhh[:,2]
    # enters the scan via d_bc when nonzero.

    # x -> (128, W*F*CC) fp16, w-major so group DMAs are contiguous
    xr = np.asarray(x[:, es]).reshape(W, E_LOC, PP, CC, F)
    xh = np.ascontiguousarray(xr.transpose(1, 2, 0, 4, 3)).reshape(128, W * F * CC)
    xh = xh.astype(np.float16)

    # diags (128, 27, 128) fp16
    pe = np.repeat(np.arange(E_LOC), PP)  # (128,) member index per partition
    dgv = np.zeros((128, NDIAG), np.float32)
    for g in range(G):
        for f in range(F):
            dgv[:, g * F + f] = Wc[pe, g, f]
        dgv[:, 24 + g] = bc[pe, g]
    dgm = np.zeros((128, NDIAG, 128), np.float16)
    idx = np.arange(128)
    dgm[idx, :, idx] = dgv.astype(np.float16)
    dgm = dgm.reshape(128, NDIAG * 128)

    # consts (128, 6+CC) f32: a, b, c, -a, -b, d, h0
    cstv = np.zeros((128, 6 + CC), np.float32)
    cstv[:, 0] = whh[es][pe, 0, 0]
    cstv[:, 1] = whh[es][pe, 1, 0]
    cstv[:, 2] = whh[es][pe, 2, 0]
    cstv[:, 3] = -cstv[:, 0]
    cstv[:, 4] = -cstv[:, 1]
    cstv[:, 5] = bhh[es][pe, 2]
    h0 = np.asarray(state[-1, es, :, 0]).reshape(E_LOC, PP, CC)
    cstv[:, 6:] = h0.reshape(128, CC)

    return {"xh": xh, "dg": dgm, "cst": cstv}


def kernel(**inputs):
    from concourse.bass_utils import run_bass_kernel_spmd

    bhh = np.asarray(inputs["bias_hh"])
    d_nonzero = bool(np.any(bhh[:, 2] != 0))

    key = ("nc", d_nonzero)
    if key not in _CACHED:
        _CACHED[key] = _build_nc(d_nonzero)
    nc = _CACHED[key]

    in_maps = [_prep_core_inputs(inputs, c) for c in range(NCORES)]
    res = run_bass_kernel_spmd(nc, in_maps, core_ids=list(range(NCORES)))

    # reassemble: per-core out (128, W*CC) -> (W, E_LOC, BI)
    full = np.zeros((W, E, B, I, 1), np.float32)
    for c in range(NCORES):
        o = np.asarray(res.results[c]["out"]).reshape(E_LOC, PP, W, CC)
        o = o.transpose(2, 0, 1, 3).reshape(W, E_LOC, BI)
        full[:, c * E_LOC : (c + 1) * E_LOC] = o.reshape(W, E_LOC, B, I, 1)
    return full


# revision 20
# speedup vs baseline: 5.8275x; 1.1914x over previous
"""EnsembleGRU Trainium2 kernel (v2).

Math (per ensemble member e, H=1):
    y  = x @ Wl^T + bl                      (proj)
    gi = y @ Wih^T + bih                    -> fold: gi = x @ Wc^T + bc
         Wc = Wih @ Wl   (3,8),  bc = Wih @ bl + bih (+ bhh for r,z gates)
    scan over W steps:
        r  = sigmoid(gi_r + a*h)            a = whh[0]
        z  = sigmoid(gi_z + b*h)            b = whh[1]
        n  = tanh(gi_n + r*(c*h + d))       c = whh[2], d = bhh[2]
        h' = (1-z)*n + z*h = q - u,  q = z*h,  u = (z-1)*n

Sharding: E=16 members over 8 cores (2 per core), zero communication.
Lane layout per core: partition p = e_loc*64 + p' (p' in 0..63),
free col c in 0..39, bi = p'*40 + c  (5120 lanes = 128 x 40).

gi is computed on the TensorEngine via accumulating diagonal matmuls
(per (gate,f) diag + per-gate bias diag vs a ones tile), into PSUM.
x is staged w-major ("p (w f c)") so each group's DMA is fully
contiguous on both sides; matmul rhs uses a strided [w, c] view.

The 64-step scan runs on DVE + ACT only.  Every DVE op is an
immediate-scalar scalar_tensor_tensor on SBUF operands (the cheapest
primitive); per-member constants (a, b, c, -a, -b, d) enter as
stride-0 broadcast in1 APs of a per-core const tile, keeping the
program SPMD-uniform.  The sigmoid is split into two [128,40] calls
(gates r and z), tanh is one [128,40] call.  Next-step gate inputs are
rebuilt from (q, u) with the P-prefetch trick so only u -> -a*u -> +P
sits on the post-tanh critical path:
    P_r(w+1) = a*q(w) + gi_r(w+1)           (in tanh's shadow)
    ar(w+1)  = (-a)*u(w) + P_r(w+1)
    h(w)     = q(w) - u(w)                  (deferred, in sigmoid's shadow)
"""

import numpy as np

W, E, B, I, F = 64, 16, 256, 10, 8
BI = B * I            # 2560
NCORES = 8
E_LOC = E // NCORES   # 2
PP = 64               # partitions per member
CC = BI // PP         # 40 free cols per step
G = 3                 # gates

WGROUPS = [8] * 8
assert sum(WGROUPS) == W
NDIAG = 27  # 24 (g,f) Wc diags + 3 bias diags

_CACHED = {}


def _build_nc(d_nonzero: bool, rep: int = 1, mm_only: bool = False,
              scan_only: bool = False, no_xdma: bool = False, act_dma: bool = False,
              dve_pad: int = 0, act_pad: int = 0, sigma_one: bool = True, aa_pad: int = 48,
              za_trick: bool = False, pad_each: bool = True,
              all_sigmoid: bool = False, pad_whole: bool = False):
    import contextlib

    import concourse.bacc as bacc
    import concourse.mybir as mybir
    from concourse.tile import TileContext

    AL = mybir.AluOpType
    AF = mybir.ActivationFunctionType
    f32 = mybir.dt.float32
    f16 = mybir.dt.float16

    nc = bacc.Bacc("TRN2", target_bir_lowering=False)

    xh = nc.dram_tensor("xh", [128, W * F * CC], f16, kind="ExternalInput")
    dg = nc.dram_tensor("dg", [128, NDIAG * 128], f16, kind="ExternalInput")
    cst = nc.dram_tensor("cst", [128, 6 + CC], f32, kind="ExternalInput")
    out = nc.dram_tensor("out", [128, W * CC], f32, kind="ExternalOutput")

    with TileContext(nc) as tc:
        with (
            tc.tile_pool(name="const", bufs=1) as constp,
            tc.tile_pool(name="xp", bufs=2) as xp,
            tc.tile_pool(name="gip", bufs=2, space="PSUM") as gip,
            tc.tile_pool(name="scan", bufs=3) as scanp,
            tc.tile_pool(name="outp", bufs=1) as outp,
        ):
            dg_sb = constp.tile([128, NDIAG * 128], f16, tag="dg")
            cst_sb = constp.tile([128, 6 + CC], f32, tag="cst")
            ones = constp.tile([128, max(WGROUPS) * CC], f16, tag="ones")
            out_sb = outp.tile([128, (W + 1) * CC], f32, tag="out")

            nc.sync.dma_start(dg_sb[:], dg[:])
            nc.sync.dma_start(cst_sb[:], cst[:])
            nc.vector.memset(ones[:], 1.0)
            # h0 into slot 0
            nc.vector.tensor_copy(out_sb[:, 0:CC], cst_sb[:, 6 : 6 + CC])

            bc = [128, CC]
            a_bc = cst_sb[:, 0:1].broadcast_to(bc)
            b_bc = cst_sb[:, 1:2].broadcast_to(bc)
            c_bc = cst_sb[:, 2:3].broadcast_to(bc)
            na_bc = cst_sb[:, 3:4].broadcast_to(bc)
            nb_bc = cst_sb[:, 4:5].broadcast_to(bc)
            d_bc = cst_sb[:, 5:6].broadcast_to(bc)
            bc2 = [128, 2, CC]
            ab_bc = cst_sb[:, 0:2].rearrange("p (g c) -> p g c", c=1).broadcast_to(bc2)
            nanb_bc = cst_sb[:, 3:5].rearrange("p (g c) -> p g c", c=1).broadcast_to(bc2)

            loop_cm = tc.For_i(0, rep, 1) if rep > 1 else contextlib.nullcontext()
            with loop_cm:
                _body(
                    nc, tc, xp, gip, scanp, xh, out, dg_sb, ones, out_sb,
                    a_bc, b_bc, c_bc, na_bc, nb_bc, d_bc, ab_bc, nanb_bc,
                    AL, AF, f32, f16,
                    d_nonzero, mm_only, scan_only, no_xdma, act_dma,
                    dve_pad, act_pad, sigma_one, all_sigmoid, pad_whole, aa_pad,
                    za_trick, pad_each,
                )

    nc.finalize()
    return nc


def _body(
    nc, tc, xp, gip, scanp, xh, out, dg_sb, ones, out_sb,
    a_bc, b_bc, c_bc, na_bc, nb_bc, d_bc, ab_bc, nanb_bc, AL, AF, f32, f16,
    d_nonzero, mm_only, scan_only, no_xdma=False, act_dma=False,
    dve_pad=0, act_pad=0, sigma_one=True, all_sigmoid=False, pad_whole=False,
    aa_pad=48, za_trick=False, pad_each=True,
):
    ngrp = len(WGROUPS)
    gstart = [sum(WGROUPS[:k]) for k in range(ngrp)]
    gi_tiles = {}

    def emit_group(k):
        WG = WGROUPS[k]
        w0 = gstart[k]
        x_t = xp.tile([128, WG * F * CC], f16, tag="x")
        if not no_xdma:
            eng = nc.scalar if (act_dma and k % 2) else nc.sync
            eng.dma_start(x_t[:], xh[:, w0 * F * CC : (w0 + WG) * F * CC])
        else:
            nc.vector.memset(x_t[:], 0.001)
        gi_ps = gip.tile([128, 3 * 512], f32, tag="gi")
        gi_tiles[k] = gi_ps
        xv = x_t[:].rearrange("p (w f c) -> p w f c", w=WG, f=F)
        if not scan_only:
            for g in range(G):
                reg = gi_ps[:, g * 512 : g * 512 + WG * CC]
                regv = reg.rearrange("p (w c) -> p w c", c=CC)
                nc.tensor.matmul(
                    reg,
                    dg_sb[:, (24 + g) * 128 : (25 + g) * 128],
                    ones[:, : WG * CC],
                    start=True,
                    stop=False,
                    skip_group_check=True,
                )
                for f in range(F):
                    nc.tensor.matmul(
                        regv,
                        dg_sb[:, (g * F + f) * 128 : (g * F + f + 1) * 128],
                        xv[:, :, f, :],
                        start=False,
                        stop=(f == F - 1),
                        skip_group_check=True,
                    )
        else:
            for g in range(G):
                nc.tensor.matmul(
                    gi_ps[:, g * 512 : g * 512 + WG * CC],
                    dg_sb[:, (24 + g) * 128 : (25 + g) * 128],
                    ones[:, : WG * CC],
                    start=True,
                    stop=True,
                    skip_group_check=True,
                )

    def gi_ap(w, g):
        k = 0
        while k + 1 < ngrp and w >= gstart[k + 1]:
            k += 1
        wl = w - gstart[k]
        return gi_tiles[k][:, g * 512 + wl * CC : g * 512 + (wl + 1) * CC]

    def gi_rz_ap(w):
        # [128, 2, CC] view of gates r,z for step w (gate stride 512)
        k = 0
        while k + 1 < ngrp and w >= gstart[k + 1]:
            k += 1
        wl = w - gstart[k]
        g3 = gi_tiles[k][:].rearrange("p (g x) -> p g x", g=3)
        return g3[:, 0:2, wl * CC : (wl + 1) * CC]

    emit_group(0)
    if ngrp > 1:
        emit_group(1)

    def emit_out_dma(k):
        nc.sync.dma_start(
            out[:, gstart[k] * CC : (gstart[k] + WGROUPS[k]) * CC],
            out_sb[:, (gstart[k] + 1) * CC : (gstart[k] + WGROUPS[k] + 1) * CC],
        )

    gends = {gstart[k] + WGROUPS[k] - 1: k for k in range(ngrp - 1)}

    if mm_only:
        for k in range(2, ngrp):
            emit_group(k)
        return

    stt = nc.vector.scalar_tensor_tensor

    # bootstrap: aa(0) = [ar|az](0) from h0, ch(0)
    AAW = 2 * CC + aa_pad
    h0 = out_sb[:, 0:CC]
    aa = scanp.tile([128, AAW], f32, tag="aa", name="aa_boot")
    ch = scanp.tile([128, CC], f32, tag="ch", name="ch_boot")
    t0 = scanp.tile([128, 2 * CC], f32, tag="t0", name="t0_boot")
    stt(t0[:, 0:CC], h0, 1.0, a_bc, AL.mult, AL.mult)
    stt(t0[:, CC:], h0, 1.0, b_bc, AL.mult, AL.mult)
    stt(aa[:, 0:CC], t0[:, 0:CC], 0.0, gi_ap(0, 0), AL.add, AL.add)
    stt(aa[:, CC : 2 * CC], t0[:, CC:], 0.0, gi_ap(0, 1), AL.add, AL.add)
    if aa_pad:
        # zero the pad lanes of this pool slot once; rotation preserves them
        nc.vector.memset(aa[:, 2 * CC :], 0.0)
    stt(ch[:], h0, 1.0, c_bc, AL.mult, AL.mult)
    if d_nonzero:
        stt(ch[:], ch[:], 0.0, d_bc, AL.add, AL.add)

    q_prev = None
    u_prev = None
    for w in range(W):
        h = out_sb[:, w * CC : (w + 1) * CC]

        rz = scanp.tile([128, AAW], f32, tag="rz")
        r_t = rz[:, 0:CC]
        z_t = rz[:, CC : 2 * CC]
        v = scanp.tile([128, CC], f32, tag="v")
        an_t = scanp.tile([128, CC], f32, tag="an")
        n_t = scanp.tile([128, CC], f32, tag="n")
        u = scanp.tile([128, CC], f32, tag="u")
        q = scanp.tile([128, CC], f32, tag="q")

        # sigmoid gates: one whole-tile call over [ar|az]
        nc.scalar.activation(rz[:], aa[:], AF.Sigmoid)
        if dve_pad or act_pad:
            pad_t = scanp.tile([128, CC], f32, tag="pad", name=f"pad_{w}")
            for _ in range(dve_pad):
                stt(pad_t[:], r_t, 1.0, a_bc, AL.mult, AL.mult)
            for _ in range(act_pad):
                nc.scalar.activation(pad_t[:], pad_t[:], AF.Sigmoid)

        # deferred h(w-1) = q - u, lands in sigmoid's shadow; ch = c*h follows
        if w >= 1:
            stt(h, q_prev, 0.0, u_prev, AL.add, AL.subtract)
            if (w - 1) in gends:
                emit_out_dma(gends[w - 1])
            ch = scanp.tile([128, CC], f32, tag="ch", name=f"ch_{w}")
            stt(ch[:], h, 1.0, c_bc, AL.mult, AL.mult)
            if d_nonzero:
                stt(ch[:], ch[:], 0.0, d_bc, AL.add, AL.add)

        # chain: v = ch * r ; an = v + gi_n(w)
        stt(v[:], ch[:], 1.0, r_t, AL.mult, AL.mult)
        stt(an_t[:], v[:], 0.0, gi_ap(w, 2), AL.add, AL.add)

        # tanh-shadow ops: q, P_r = a*q + gi_r(w+1), P_z = b*q + gi_z(w+1)
        stt(q[:], z_t, 1.0, h, AL.mult, AL.mult)
        if za_trick and w + 1 < W:
            zab = scanp.tile([128, 2 * CC], f32, tag="zab", name=f"zab_{w}")
            stt(zab[:, 0:CC], z_t, 1.0, na_bc, AL.subtract, AL.mult)
            stt(zab[:, CC:], z_t, 1.0, nb_bc, AL.subtract, AL.mult)
        if w + 1 < W:
            aa_n = scanp.tile([128, AAW], f32, tag="aa", name=f"aa_{w}")
            pp_t = scanp.tile([128, 2 * CC], f32, tag="pp", name=f"pp_{w}")
            t_n = scanp.tile([128, 2 * CC], f32, tag="t0", name=f"t_{w}")
            stt(t_n[:, 0:CC], q[:], 1.0, a_bc, AL.mult, AL.mult)
            stt(pp_t[:, 0:CC], t_n[:, 0:CC], 0.0, gi_ap(w + 1, 0), AL.add, AL.add)
            stt(t_n[:, CC:], q[:], 1.0, b_bc, AL.mult, AL.mult)
            stt(pp_t[:, CC:], t_n[:, CC:], 0.0, gi_ap(w + 1, 1), AL.add, AL.add)

        nc.scalar.activation(n_t[:], an_t[:],
                             AF.Sigmoid if all_sigmoid else AF.Tanh)

        # chain: u = (z-1)*n ; ar = -a*u + P_r ; az = -b*u + P_z
        stt(u[:], z_t, 1.0, n_t[:], AL.subtract, AL.mult)
        if w + 1 < W:
            if za_trick:
                # -a*u = ((z-1)*(-a)) * n, computed straight from n
                stt(t_n[:, 0:CC], n_t[:], 1.0, zab[:, 0:CC], AL.mult, AL.mult)
            else:
                stt(t_n[:, 0:CC], u[:], 1.0, na_bc, AL.mult, AL.mult)
            stt(aa_n[:, 0:CC], t_n[:, 0:CC], 0.0, pp_t[:, 0:CC], AL.add, AL.add)
            if za_trick:
                stt(t_n[:, CC:], n_t[:], 1.0, zab[:, CC:], AL.mult, AL.mult)
            else:
                stt(t_n[:, CC:], u[:], 1.0, nb_bc, AL.mult, AL.mult)
            stt(aa_n[:, CC : 2 * CC], t_n[:, CC:], 0.0, pp_t[:, CC:], AL.add, AL.add)
            if aa_pad and (pad_each or w < 2):
                nc.vector.memset(aa_n[:, 2 * CC :], 0.0)
            aa = aa_n

        q_prev, u_prev = q, u

        # interleave: after the first step of group k, emit group k+2
        k = 0
        while k + 1 < ngrp and w >= gstart[k + 1]:
            k += 1
        if w == gstart[k] and k + 2 < ngrp:
            emit_group(k + 2)

    # final h(W-1) and last group's output
    stt(out_sb[:, W * CC : (W + 1) * CC], q_prev, 0.0, u_prev, AL.add, AL.subtract)
    emit_out_dma(ngrp - 1)


def _prep_core_inputs(inputs, core):
    x = inputs["inputs"]          # (W,E,B,I,F) f32
    state = inputs["state"]       # (1,E,BI,1)
    wl = inputs["weight_linear"]  # (E,16,F)
    bl = inputs["bias_linear"]    # (E,16)
    wih = inputs["weight_ih"]     # (E,3,16)
    whh = inputs["weight_hh"]     # (E,3,1)
    bih = inputs["bias_ih"]       # (E,3)
    bhh = inputs["bias_hh"]       # (E,3)

    es = slice(core * E_LOC, (core + 1) * E_LOC)
    # fold weights
    Wc = np.einsum("egp,epf->egf", wih[es], wl[es])          # (2,3,F)
    bc = np.einsum("egp,ep->eg", wih[es], bl[es]) + bih[es]  # (2,3)
    bc = bc.copy()
    bc[:, 0] += bhh[es][:, 0]
    bc[:, 1] += bhh[es][:, 1]
    # gate-n bias (bc[:,2]) included in the bias diag matmul; d=bhh[:,2]
    # enters the scan via d_bc when nonzero.

    # x -> (128, W*F*CC) fp16, w-major so group DMAs are contiguous
    xr = np.asarray(x[:, es]).reshape(W, E_LOC, PP, CC, F)
    xh = np.ascontiguousarray(xr.transpose(1, 2, 0, 4, 3)).reshape(128, W * F * CC)
    xh = xh.astype(np.float16)

    # diags (128, 27, 128) fp16
    pe = np.repeat(np.arange(E_LOC), PP)  # (128,) member index per partition
    dgv = np.zeros((128, NDIAG), np.float32)
    for g in range(G):
        for f in range(F):
            dgv[:, g * F + f] = Wc[pe, g, f]
        dgv[:, 24 + g] = bc[pe, g]
    dgm = np.zeros((128, NDIAG, 128), np.float16)
    idx = np.arange(128)
    dgm[idx, :, idx] = dgv.astype(np.float16)
    dgm = dgm.reshape(128, NDIAG * 128)

    # consts (128, 6+CC) f32: a, b, c, -a, -b, d, h0
    cstv = np.zeros((128, 6 + CC), np.float32)
    cstv[:, 0] = whh[es][pe, 0, 0]
    cstv[:, 1] = whh[es][pe, 1, 0]
    cstv[:, 2] = whh[es][pe, 2, 0]
    cstv[:, 3] = -cstv[:, 0]
    cstv[:, 4] = -cstv[:, 1]
    cstv[:, 5] = bhh[es][pe, 2]
    h0 = np.asarray(state[-1, es, :, 0]).reshape(E_LOC, PP, CC)
    cstv[:, 6:] = h0.reshape(128, CC)

    return {"xh": xh, "dg": dgm, "cst": cstv}


def kernel(**inputs):
    from concourse.bass_utils import run_bass_kernel_spmd

    bhh = np.asarray(inputs["bias_hh"])
    d_nonzero = bool(np.any(bhh[:, 2] != 0))

    key = ("nc", d_nonzero)
    if key not in _CACHED:
        _CACHED[key] = _build_nc(d_nonzero)
    nc = _CACHED[key]

    in_maps = [_prep_core_inputs(inputs, c) for c in range(NCORES)]
    res = run_bass_kernel_spmd(nc, in_maps, core_ids=list(range(NCORES)))

    # reassemble: per-core out (128, W*CC) -> (W, E_LOC, BI)
    full = np.zeros((W, E, B, I, 1), np.float32)
    for c in range(NCORES):
        o = np.asarray(res.results[c]["out"]).reshape(E_LOC, PP, W, CC)
        o = o.transpose(2, 0, 1, 3).reshape(W, E_LOC, BI)
        full[:, c * E_LOC : (c + 1) * E_LOC] = o.reshape(W, E_LOC, B, I, 1)
    return full
